# revision 6
# baseline (speedup 1.0000x reference)
"""AxialAttention Trainium2 kernel (8 NeuronCores, SPMD) — v2.

Sharding: core = b*4 + q; each core: one batch element, 10-row H-slab, all 256
channels. out = 3 * branch (branches identical); factor 3 folded into wp,
attention scale folded into wq/bq.

v2 vs v1: the q/k/v pivot transposes (PE, 307k cyc) and their psum->SBUF
copies (Act/DVE) are replaced by XBAR DMA transposes (InstDmaTransposeAnt,
14ns/tile on the mostly-idle DMA engines). The conv evacuates q,k into
w-major 48-padded slabs and v into a d-major 48-padded slab (the xbar
semantics out[p,m,l] = in[l, m*48+p] make the pad columns land on junk
partitions 40-47/104-111). Attention reads the same [d|w' @ deck-band,
(w|d, c)] layout as v1. AV psum is bf16 (2x DVE evac); output store is bf16.
"""

import sys

sys.path.insert(0, "/opt/trn_rl_repo")

import numpy as np
import ml_dtypes
from contextlib import ExitStack

import concourse.bass as bass
import concourse.tile as tile
from concourse import bacc, mybir
from concourse.bass_utils import run_bass_kernel_spmd
from concourse.masks import make_identity

BF16 = mybir.dt.bfloat16
F32 = mybir.dt.float32

B, C, H, W, D = 2, 256, 40, 40, 40
HEADS = 8
HD = C // HEADS
SCALE = HD ** -0.5
N_CORES = 8
SLAB = H // 4          # 10 H-rows per core
WD = W * D             # 1600
NSLAB = SLAB * WD      # 16000
CHALF = 128
PADW = 48              # xbar pad: free = 40 * 48, junk -> partitions 40-47


def _merge(a, b):
    """Proportionally interleave two chunk lists, preserving each order."""
    out = []
    na, nb = len(a), len(b)
    ia = ib = 0
    while ia < na or ib < nb:
        if ib >= nb or (ia * (nb + 1) <= ib * (na + 1) and ia < na):
            out.append(a[ia])
            ia += 1
        else:
            out.append(b[ib])
            ib += 1
    return out


def _build_nc():
    nc = bacc.Bacc(
        "TRN2",
        target_bir_lowering=False,
        debug=False,
        num_devices=N_CORES,
    )
    x_d = nc.declare_dram_parameter("x", [C, NSLAB], BF16, isOutput=False)
    wqkv_d = nc.declare_dram_parameter("wqkv", [C, 3 * C], BF16, isOutput=False)
    bqkv_d = nc.declare_dram_parameter("bqkv", [3 * C, 1], F32, isOutput=False)
    wp_d = nc.declare_dram_parameter("wp3", [C, C], BF16, isOutput=False)
    bp_d = nc.declare_dram_parameter("bp", [C, 1], F32, isOutput=False)
    out_d = nc.declare_dram_parameter("out", [C, NSLAB], BF16, isOutput=True)

    IDENT = mybir.ActivationFunctionType.Identity
    EXP = mybir.ActivationFunctionType.Exp
    MULT = mybir.AluOpType.mult

    with ExitStack() as ctx:
        tc = ctx.enter_context(tile.TileContext(nc))
        const = ctx.enter_context(tc.tile_pool(name="const", bufs=1))
        xp = ctx.enter_context(tc.tile_pool(name="xp", bufs=2))
        attp = ctx.enter_context(tc.tile_pool(name="attp", bufs=2))
        ep = ctx.enter_context(tc.tile_pool(name="ep", bufs=3))
        oallp = ctx.enter_context(tc.tile_pool(name="oallp", bufs=2))
        brp = ctx.enter_context(tc.tile_pool(name="brp", bufs=2))
        recp = ctx.enter_context(tc.tile_pool(name="recp", bufs=4))
        outp = ctx.enter_context(tc.tile_pool(name="outp", bufs=2))
        # conv+wp 2-bank tiles; scores tiles; o_ps/pb share the last 2 banks
        ps_cs = ctx.enter_context(tc.tile_pool(name="ps_cs", bufs=2, space="PSUM"))
        ps_s = ctx.enter_context(tc.tile_pool(name="ps_s", bufs=2, space="PSUM"))
        ps_x = ctx.enter_context(tc.tile_pool(name="ps_x", bufs=2, space="PSUM"))
        ps_t = ctx.enter_context(tc.tile_pool(name="ps_t", bufs=2, space="PSUM"))

        ident = const.tile([128, 128], BF16)
        make_identity(nc, ident[:])

        def load_x(i):
            x_sb = xp.tile([128, 2, WD], BF16, name="x_sb")
            nc.sync.dma_start(
                x_sb[:],
                x_d.ap()[:, i * WD : (i + 1) * WD].rearrange(
                    "(ko ki) n -> ki ko n", ki=128
                ),
            )
            return x_sb

        # DMA order: x0 and wqkv first so the first conv starts ASAP
        x0 = load_x(0)
        wqkv_sb = const.tile([128, 2, 3 * C], BF16)
        nc.sync.dma_start(
            wqkv_sb[:, :, 0:256],
            wqkv_d.ap()[:, 0:256].rearrange("(ko ki) m -> ki ko m", ki=128),
        )
        bqkv_sb = const.tile([128, 6, 1], F32)
        nc.sync.dma_start(
            bqkv_sb[:], bqkv_d.ap().rearrange("(mo mi) one -> mi mo one", mi=128)
        )
        x1 = load_x(1)
        nc.sync.dma_start(
            wqkv_sb[:, :, 256 : 3 * C],
            wqkv_d.ap()[:, 256 : 3 * C].rearrange(
                "(ko ki) m -> ki ko m", ki=128
            ),
        )
        wp_sb = const.tile([128, 2, C], BF16)
        nc.sync.dma_start(
            wp_sb[:], wp_d.ap().rearrange("(ko ki) m -> ki ko m", ki=128)
        )
        bp_sb = const.tile([128, 2, 1], F32)
        nc.sync.dma_start(
            bp_sb[:], bp_d.ap().rearrange("(mo mi) one -> mi mo one", mi=128)
        )

        # persistent double-buffered compact qkv slabs (w-major), flat with a
        # 1KB pad so 64-wide pivot reads (M=64 fills psum rows fully; the PE
        # cost is out-free-size, so the widening is free) stay in-bounds
        qkv_slab0 = const.tile([128, 6 * WD + 1024], BF16)
        qkv_slab1 = const.tile([128, 6 * WD + 1024], BF16)
        qkv_bufs = (qkv_slab0, qkv_slab1)
        nc.vector.memset(qkv_slab0[:, 6 * WD :], 0.0)
        nc.vector.memset(qkv_slab1[:, 6 * WD :], 0.0)


        # channel-pair groups (deck covers c_local and c_local+128)
        groups = []
        c0 = 0
        while c0 < CHALF:
            groups.append((c0, min(12, CHALF - c0)))
            c0 += 12

        def conv_emit(x_sb, qkv_sb):
            # qkv slab: [128, m(6), 40, 48] padded; q,k w-major (outer=w),
            # v d-major (outer=d). m: q0,q1,k0,k1,v0,v1 (out-channel decks).
            chunks = []
            for m in range(6):
                for n2 in range(2):
                    def ch(m=m, n2=n2):
                        for g in range(2):
                            n = n2 * 2 + g
                            pt = ps_cs.tile(
                                [128, 512], F32, tag="ps_cs", name="conv_ps"
                            )
                            ps = pt[:, 0:400]
                            for k in range(2):
                                nc.tensor.matmul(
                                    ps[:],
                                    lhsT=wqkv_sb[:, k, m * 128 : (m + 1) * 128],
                                    rhs=x_sb[:, k, n * 400 : (n + 1) * 400],
                                    start=(k == 0),
                                    stop=(k == 1),
                                )
                            dst = qkv_sb[
                                :, m * WD + n * 400 : m * WD + (n + 1) * 400
                            ]
                            if (m + n2) % 3 == 0:
                                nc.vector.tensor_scalar_add(
                                    dst, ps[:], bqkv_sb[:, m]
                                )
                            else:
                                nc.scalar.activation(
                                    out=dst,
                                    in_=ps[:],
                                    func=IDENT,
                                    bias=bqkv_sb[:, m],
                                    scale=1.0,
                                )
                    chunks.append(ch)
            return chunks

        def xbar_emit(qkv_sb, parity):
            # PE-transpose pivots: q/k/v [d|w' @ deck-band, (w|d, c)].
            # pst is bf16 psum, evacuated with DVE 2x or Act.
            q_att = attp.tile([128, W * CHALF], BF16, tag="q_att", name="q_att")
            k_att = attp.tile([128, W * CHALF], BF16, tag="k_att", name="k_att")
            v_att = attp.tile([128, 41 * CHALF], BF16, tag="v_att", name="v_att")
            v_view = qkv_sb[:, 0 : 6 * WD].rearrange(
                "p (m w d) -> p m d w", m=6, d=D
            )
            chunks = [
                lambda: nc.vector.memset(v_att[:, 40 * CHALF : 41 * CHALF], 1.0)
            ]
            for src, dst, eng in (
                (0, q_att, "v"),
                (2, k_att, "s"),
                (4, v_att, "v"),
            ):
                for wg in range(5):
                    def ch(wg=wg, src=src, dst=dst, eng=eng):
                        pst = ps_t.tile(
                            [128, 1024], BF16, tag="ps_t", name="pst"
                        )
                        for wl in range(8):
                            w = wg * 8 + wl
                            for cc in range(2):
                                r0 = cc * 64
                                if src == 4:
                                    in_ap = v_view[:, 4 + cc, w]
                                else:
                                    off = (src + cc) * WD + w * 40
                                    in_ap = qkv_sb[:, off : off + 40]
                                nc.tensor.transpose(
                                    pst[r0 : r0 + 40, wl * 128 : (wl + 1) * 128],
                                    in_ap,
                                    ident[:],
                                )
                        if eng == "s":
                            nc.scalar.copy(
                                dst[0:104, wg * 1024 : (wg + 1) * 1024],
                                pst[0:104, :],
                            )
                        else:
                            nc.vector.tensor_copy(
                                out=dst[0:104, wg * 1024 : (wg + 1) * 1024],
                                in_=pst[0:104, :],
                            )
                    chunks.append(ch)
            return (q_att, k_att, v_att), chunks

        def attn_emit(att):
            q_att, k_att, v_att = att
            k_v = k_att.rearrange("p (w c) -> p c w", c=CHALF)
            q_v = q_att.rearrange("p (w c) -> p c w", c=CHALF)
            vv = v_att.rearrange("p (d c) -> p c d", c=CHALF)
            o_all = oallp.tile([128, CHALF * W], BF16, name="o_all")

            def scores_stage(c0, gn):
                s_ps = ps_s.tile([128, 512], F32, tag="ps_s", name="s_ps")
                for j in range(gn):
                    for cc in range(2):
                        r0 = cc * 64
                        nc.tensor.matmul(
                            s_ps[r0 : r0 + 40, j * 40 : (j + 1) * 40],
                            lhsT=k_v[r0 : r0 + 40, c0 + j],
                            rhs=q_v[r0 : r0 + 40, c0 + j],
                            start=True,
                            stop=True,
                        )
                e_sb = ep.tile([128, 480], BF16, tag="e_sb", name="e_sb")
                nc.scalar.activation(
                    out=e_sb[0:104, : gn * 40], in_=s_ps[0:104, : gn * 40], func=EXP
                )
                return e_sb

            def av_stage(c0, gn, e_sb, alt=[0]):
                o_ps = ps_x.tile([128, 512], F32, tag="ps_x", name="o_ps")
                for j in range(gn):
                    for cc in range(2):
                        r0 = cc * 64
                        nc.tensor.matmul(
                            o_ps[r0 : r0 + 41, j * 40 : (j + 1) * 40],
                            lhsT=vv[r0 : r0 + 40, c0 + j],
                            rhs=e_sb[r0 : r0 + 40, j * 40 : (j + 1) * 40],
                            start=True,
                            stop=True,
                        )
                nc.vector.tensor_copy(
                    out=o_all[0:105, c0 * 40 : c0 * 40 + gn * 40],
                    in_=o_ps[0:105, : gn * 40],
                )

            pend = [None]
            chunks = []
            for c0, gn in groups:
                def ch(c0=c0, gn=gn):
                    e_sb = scores_stage(c0, gn)
                    if pend[0] is not None:
                        av_stage(*pend[0])
                    pend[0] = (c0, gn, e_sb)
                chunks.append(ch)
            chunks.append(lambda: av_stage(*pend[0]))
            return o_all, chunks

        def pbwp_emit(o_all, i):
            branch_sb = brp.tile([128, 2, WD], BF16, name="branch_sb")
            out_sb = outp.tile([128, 2, WD], BF16, name="out_sb")
            o_v = o_all.rearrange("p (c w) -> p w c", w=W)
            chunks = []
            for wb in range(5):
                def ch(wb=wb):
                    # two per-deck transposes per w (skip junk rows 41-63:
                    # transpose cost = input partition count)
                    pb_full = ps_x.tile([128, 1024], BF16, tag="ps_x", name="pb")
                    pb = pb_full[:, 0:672]
                    for wl in range(8):
                        w = wb * 8 + wl
                        nc.tensor.transpose(
                            pb[:, wl * 84 : wl * 84 + 41],
                            o_v[0:41, w, :],
                            ident[0:41, 0:41],
                        )
                        nc.tensor.transpose(
                            pb[:, wl * 84 + 42 : wl * 84 + 83],
                            o_v[64:105, w, :],
                            ident[64:105, 64:105],
                        )
                    pb_v = pb.rearrange("p (w q) -> p w q", q=84)
                    rec = recp.tile([128, 8, 2], F32, tag="rec", name="rec")
                    nc.vector.reciprocal(rec[:, :, 0], pb_v[:, :, 40])
                    nc.vector.reciprocal(rec[:, :, 1], pb_v[:, :, 82])
                    for cc in range(2):
                        nc.vector.tensor_tensor(
                            branch_sb[:, cc].rearrange("p (w d) -> p w d", d=40)[
                                :, wb * 8 : wb * 8 + 8
                            ],
                            pb_v[:, :, cc * 42 : cc * 42 + 40],
                            rec[:, :, cc : cc + 1].to_broadcast((128, 8, 40)),
                            MULT,
                        )
                chunks.append(ch)
            for m in range(2):
                for n2 in range(2):
                    def ch(m=m, n2=n2):
                        for g in range(2):
                            n = n2 * 2 + g
                            pt = ps_cs.tile(
                                [128, 512], F32, tag="ps_cs", name="wp_ps"
                            )
                            ps = pt[:, 0:400]
                            for k in range(2):
                                nc.tensor.matmul(
                                    ps[:],
                                    lhsT=wp_sb[:, k, m * 128 : (m + 1) * 128],
                                    rhs=branch_sb[:, k, n * 400 : (n + 1) * 400],
                                    start=(k == 0),
                                    stop=(k == 1),
                                )
                            nc.scalar.activation(
                                out=out_sb[:, m, n * 400 : (n + 1) * 400],
                                in_=ps[:],
                                func=IDENT,
                                bias=bp_sb[:, m],
                                scale=1.0,
                            )
                    chunks.append(ch)

            out_dv = out_d.ap()[:, i * WD : (i + 1) * WD].rearrange(
                "(ko ki) n -> ki ko n", ki=128
            )
            if i == SLAB - 1:
                # split the final store so the m=0 half overlaps m=1's evacs
                def dma_m(m):
                    nc.scalar.dma_start(out_dv[:, m : m + 1], out_sb[:, m : m + 1])

                chunks.insert(7, lambda: dma_m(0))
                chunks.append(lambda: dma_m(1))
            else:
                def dma_ch():
                    nc.scalar.dma_start(out_dv, out_sb[:])

                chunks.append(dma_ch)
            return chunks

        # ---- software-pipelined slice loop, conv two rows ahead:
        # iter i runs attn(i) | conv(i+2) | pbwp(i-1), with xbar(i+1)
        # issued at iter start (its qkv slab was filled during iter i-1,
        # so the transpose has a full row of slack before attn(i+1))
        conv_ch = conv_emit(x0, qkv_bufs[0])
        att_cur, piv0_ch = xbar_emit(qkv_bufs[0], 0)
        for ch in conv_ch:
            ch()
        for ch in _merge(piv0_ch, conv_emit(x1, qkv_bufs[1])):
            ch()
        x2 = load_x(2)
        _x_hold = [x2]
        pending_pbwp = []
        att_nxt = None
        for i in range(SLAB):
            if i + 1 < SLAB:
                att_nxt, piv_ch = xbar_emit(qkv_bufs[(i + 1) % 2], (i + 1) % 2)
            else:
                att_nxt, piv_ch = None, []
            if i + 2 < SLAB:
                x_nxt = _x_hold[0]
                if i + 3 < SLAB:
                    _x_hold[0] = load_x(i + 3)
                other = conv_emit(x_nxt, qkv_bufs[(i + 2) % 2])
            else:
                other = []
            o_all, attn_ch = attn_emit(att_cur)
            # pbwp early (inputs ready at row start), conv late; pivots of the
            # next row spread throughout
            rest = _merge(piv_ch, list(pending_pbwp) + list(other))
            skew = min(2, len(rest))
            for ch in rest[:skew]:
                ch()
            for ch in _merge(attn_ch, rest[skew:]):
                ch()
            pending_pbwp = pbwp_emit(o_all, i)
            att_cur = att_nxt
        for ch in pending_pbwp:
            ch()

    nc.compile()
    return nc


_NC_CACHE = None


def _get_nc():
    global _NC_CACHE
    if _NC_CACHE is None:
        _NC_CACHE = _build_nc()
    return _NC_CACHE


def make_in_maps(x, wq, bq, wk, bk, wv, bv, wp, bp):
    bf = ml_dtypes.bfloat16
    wqkv = np.concatenate(
        [wq.T * SCALE, wk.T, wv.T], axis=1
    ).astype(bf)  # [C, 3C], lhsT layout (c_in rows, c_out cols)
    bqkv = np.concatenate([bq * SCALE, bk, bv]).reshape(3 * C, 1).astype(np.float32)
    wp3 = (3.0 * wp).T.astype(bf)  # [C, C]
    bp_ = bp.reshape(C, 1).astype(np.float32)
    in_maps = []
    for core in range(N_CORES):
        b = core // 4
        r0 = (core % 4) * SLAB
        x_slab = np.ascontiguousarray(
            x[b, :, r0 : r0 + SLAB].reshape(C, NSLAB)
        ).astype(bf)
        in_maps.append(
            {"x": x_slab, "wqkv": wqkv, "bqkv": bqkv, "wp3": wp3, "bp": bp_}
        )
    return in_maps


def run_on_cores(in_maps, **kw):
    nc = _get_nc()
    return run_bass_kernel_spmd(nc, in_maps, core_ids=list(range(N_CORES)), **kw)


def kernel(x, wq, bq, wk, bk, wv, bv, wp, bp):
    x = np.asarray(x, dtype=np.float32)
    in_maps = make_in_maps(
        x,
        np.asarray(wq, np.float32),
        np.asarray(bq, np.float32),
        np.asarray(wk, np.float32),
        np.asarray(bk, np.float32),
        np.asarray(wv, np.float32),
        np.asarray(bv, np.float32),
        np.asarray(wp, np.float32),
        np.asarray(bp, np.float32),
    )
    res = run_on_cores(in_maps)
    out = np.empty((B, C, H, W, D), np.float32)
    for core in range(N_CORES):
        b = core // 4
        r0 = (core % 4) * SLAB
        out[b, :, r0 : r0 + SLAB] = (
            res.results[core]["out"].astype(np.float32).reshape(C, SLAB, W, D)
        )
    return out


# revision 7
# speedup vs baseline: 1.0067x; 1.0067x over previous
"""AxialAttention Trainium2 kernel (8 NeuronCores, SPMD) — v2.

Sharding: core = b*4 + q; each core: one batch element, 10-row H-slab, all 256
channels. out = 3 * branch (branches identical); factor 3 folded into wp,
attention scale folded into wq/bq.

v2 vs v1: the q/k/v pivot transposes (PE, 307k cyc) and their psum->SBUF
copies (Act/DVE) are replaced by XBAR DMA transposes (InstDmaTransposeAnt,
14ns/tile on the mostly-idle DMA engines). The conv evacuates q,k into
w-major 48-padded slabs and v into a d-major 48-padded slab (the xbar
semantics out[p,m,l] = in[l, m*48+p] make the pad columns land on junk
partitions 40-47/104-111). Attention reads the same [d|w' @ deck-band,
(w|d, c)] layout as v1. AV psum is bf16 (2x DVE evac); output store is bf16.
"""

import sys

sys.path.insert(0, "/opt/trn_rl_repo")

import numpy as np
import ml_dtypes
from contextlib import ExitStack

import concourse.bass as bass
import concourse.tile as tile
from concourse import bacc, mybir
from concourse.bass_utils import run_bass_kernel_spmd
from concourse.masks import make_identity

BF16 = mybir.dt.bfloat16
F32 = mybir.dt.float32

B, C, H, W, D = 2, 256, 40, 40, 40
HEADS = 8
HD = C // HEADS
SCALE = HD ** -0.5
N_CORES = 8
SLAB = H // 4          # 10 H-rows per core
WD = W * D             # 1600
NSLAB = SLAB * WD      # 16000
CHALF = 128
PADW = 48              # xbar pad: free = 40 * 48, junk -> partitions 40-47


def _merge(a, b):
    """Proportionally interleave two chunk lists, preserving each order."""
    out = []
    na, nb = len(a), len(b)
    ia = ib = 0
    while ia < na or ib < nb:
        if ib >= nb or (ia * (nb + 1) <= ib * (na + 1) and ia < na):
            out.append(a[ia])
            ia += 1
        else:
            out.append(b[ib])
            ib += 1
    return out


def _build_nc():
    nc = bacc.Bacc(
        "TRN2",
        target_bir_lowering=False,
        debug=False,
        num_devices=N_CORES,
    )
    x_d = nc.declare_dram_parameter("x", [C, NSLAB], BF16, isOutput=False)
    wqkv_d = nc.declare_dram_parameter("wqkv", [C, 3 * C], BF16, isOutput=False)
    bqkv_d = nc.declare_dram_parameter("bqkv", [3 * C, 1], F32, isOutput=False)
    wp_d = nc.declare_dram_parameter("wp3", [C, C], BF16, isOutput=False)
    bp_d = nc.declare_dram_parameter("bp", [C, 1], F32, isOutput=False)
    out_d = nc.declare_dram_parameter("out", [C, NSLAB], BF16, isOutput=True)

    IDENT = mybir.ActivationFunctionType.Identity
    EXP = mybir.ActivationFunctionType.Exp
    MULT = mybir.AluOpType.mult

    with ExitStack() as ctx:
        tc = ctx.enter_context(tile.TileContext(nc))
        const = ctx.enter_context(tc.tile_pool(name="const", bufs=1))
        xp = ctx.enter_context(tc.tile_pool(name="xp", bufs=2))
        attp = ctx.enter_context(tc.tile_pool(name="attp", bufs=2))
        ep = ctx.enter_context(tc.tile_pool(name="ep", bufs=3))
        oallp = ctx.enter_context(tc.tile_pool(name="oallp", bufs=2))
        brp = ctx.enter_context(tc.tile_pool(name="brp", bufs=2))
        recp = ctx.enter_context(tc.tile_pool(name="recp", bufs=4))
        outp = ctx.enter_context(tc.tile_pool(name="outp", bufs=2))
        # conv+wp 2-bank tiles; scores tiles; o_ps/pb share the last 2 banks
        ps_cs = ctx.enter_context(tc.tile_pool(name="ps_cs", bufs=2, space="PSUM"))
        ps_s = ctx.enter_context(tc.tile_pool(name="ps_s", bufs=2, space="PSUM"))
        ps_x = ctx.enter_context(tc.tile_pool(name="ps_x", bufs=2, space="PSUM"))
        ps_t = ctx.enter_context(tc.tile_pool(name="ps_t", bufs=2, space="PSUM"))

        ident = const.tile([128, 128], BF16)
        make_identity(nc, ident[:])

        def load_x(i):
            x_sb = xp.tile([128, 2, WD], BF16, name="x_sb")
            nc.sync.dma_start(
                x_sb[:],
                x_d.ap()[:, i * WD : (i + 1) * WD].rearrange(
                    "(ko ki) n -> ki ko n", ki=128
                ),
            )
            return x_sb

        # DMA order: x0 (split halves) and wqkv-q first so conv starts ASAP
        x0 = xp.tile([128, 2, WD], BF16, name="x_sb")
        x0_src = x_d.ap()[:, 0:WD].rearrange("(ko ki) n -> ki ko n", ki=128)
        nc.sync.dma_start(x0[:, :, 0:800], x0_src[:, :, 0:800])
        wqkv_sb = const.tile([128, 2, 3 * C], BF16)
        nc.sync.dma_start(
            wqkv_sb[:, :, 0:256],
            wqkv_d.ap()[:, 0:256].rearrange("(ko ki) m -> ki ko m", ki=128),
        )
        bqkv_sb = const.tile([128, 6, 1], F32)
        nc.sync.dma_start(
            bqkv_sb[:], bqkv_d.ap().rearrange("(mo mi) one -> mi mo one", mi=128)
        )
        nc.sync.dma_start(x0[:, :, 800:WD], x0_src[:, :, 800:WD])
        x1 = load_x(1)
        nc.sync.dma_start(
            wqkv_sb[:, :, 256 : 3 * C],
            wqkv_d.ap()[:, 256 : 3 * C].rearrange(
                "(ko ki) m -> ki ko m", ki=128
            ),
        )
        wp_sb = const.tile([128, 2, C], BF16)
        nc.sync.dma_start(
            wp_sb[:], wp_d.ap().rearrange("(ko ki) m -> ki ko m", ki=128)
        )
        bp_sb = const.tile([128, 2, 1], F32)
        nc.sync.dma_start(
            bp_sb[:], bp_d.ap().rearrange("(mo mi) one -> mi mo one", mi=128)
        )

        # persistent double-buffered compact qkv slabs (w-major), flat with a
        # 1KB pad so 64-wide pivot reads (M=64 fills psum rows fully; the PE
        # cost is out-free-size, so the widening is free) stay in-bounds
        qkv_slab0 = const.tile([128, 6 * WD + 1024], BF16)
        qkv_slab1 = const.tile([128, 6 * WD + 1024], BF16)
        qkv_bufs = (qkv_slab0, qkv_slab1)
        nc.vector.memset(qkv_slab0[:, 6 * WD :], 0.0)
        nc.vector.memset(qkv_slab1[:, 6 * WD :], 0.0)


        # channel-pair groups (deck covers c_local and c_local+128)
        groups = []
        c0 = 0
        while c0 < CHALF:
            groups.append((c0, min(12, CHALF - c0)))
            c0 += 12

        def conv_emit(x_sb, qkv_sb):
            # qkv slab: [128, m(6), 40, 48] padded; q,k w-major (outer=w),
            # v d-major (outer=d). m: q0,q1,k0,k1,v0,v1 (out-channel decks).
            chunks = []
            for m in range(6):
                for n2 in range(2):
                    def ch(m=m, n2=n2):
                        for g in range(2):
                            n = n2 * 2 + g
                            pt = ps_cs.tile(
                                [128, 512], F32, tag="ps_cs", name="conv_ps"
                            )
                            ps = pt[:, 0:400]
                            for k in range(2):
                                nc.tensor.matmul(
                                    ps[:],
                                    lhsT=wqkv_sb[:, k, m * 128 : (m + 1) * 128],
                                    rhs=x_sb[:, k, n * 400 : (n + 1) * 400],
                                    start=(k == 0),
                                    stop=(k == 1),
                                )
                            dst = qkv_sb[
                                :, m * WD + n * 400 : m * WD + (n + 1) * 400
                            ]
                            if (m + n2) % 3 == 0:
                                nc.vector.tensor_scalar_add(
                                    dst, ps[:], bqkv_sb[:, m]
                                )
                            else:
                                nc.scalar.activation(
                                    out=dst,
                                    in_=ps[:],
                                    func=IDENT,
                                    bias=bqkv_sb[:, m],
                                    scale=1.0,
                                )
                    chunks.append(ch)
            return chunks

        def xbar_emit(qkv_sb, parity):
            # PE-transpose pivots: q/k/v [d|w' @ deck-band, (w|d, c)].
            # pst is bf16 psum, evacuated with DVE 2x or Act.
            q_att = attp.tile([128, W * CHALF], BF16, tag="q_att", name="q_att")
            k_att = attp.tile([128, W * CHALF], BF16, tag="k_att", name="k_att")
            v_att = attp.tile([128, 41 * CHALF], BF16, tag="v_att", name="v_att")
            v_view = qkv_sb[:, 0 : 6 * WD].rearrange(
                "p (m w d) -> p m d w", m=6, d=D
            )
            chunks = [
                lambda: nc.vector.memset(v_att[:, 40 * CHALF : 41 * CHALF], 1.0)
            ]
            for src, dst, eng in (
                (0, q_att, "v"),
                (2, k_att, "s"),
                (4, v_att, "v"),
            ):
                for wg in range(5):
                    def ch(wg=wg, src=src, dst=dst, eng=eng):
                        pst = ps_t.tile(
                            [128, 1024], BF16, tag="ps_t", name="pst"
                        )
                        for wl in range(8):
                            w = wg * 8 + wl
                            for cc in range(2):
                                r0 = cc * 64
                                if src == 4:
                                    in_ap = v_view[:, 4 + cc, w]
                                else:
                                    off = (src + cc) * WD + w * 40
                                    in_ap = qkv_sb[:, off : off + 40]
                                nc.tensor.transpose(
                                    pst[r0 : r0 + 40, wl * 128 : (wl + 1) * 128],
                                    in_ap,
                                    ident[:],
                                )
                        if eng == "s":
                            nc.scalar.copy(
                                dst[0:104, wg * 1024 : (wg + 1) * 1024],
                                pst[0:104, :],
                            )
                        else:
                            nc.vector.tensor_copy(
                                out=dst[0:104, wg * 1024 : (wg + 1) * 1024],
                                in_=pst[0:104, :],
                            )
                    chunks.append(ch)
            return (q_att, k_att, v_att), chunks

        def attn_emit(att):
            q_att, k_att, v_att = att
            k_v = k_att.rearrange("p (w c) -> p c w", c=CHALF)
            q_v = q_att.rearrange("p (w c) -> p c w", c=CHALF)
            vv = v_att.rearrange("p (d c) -> p c d", c=CHALF)
            o_all = oallp.tile([128, CHALF * W], BF16, name="o_all")

            def scores_stage(c0, gn):
                s_ps = ps_s.tile([128, 512], F32, tag="ps_s", name="s_ps")
                for j in range(gn):
                    for cc in range(2):
                        r0 = cc * 64
                        nc.tensor.matmul(
                            s_ps[r0 : r0 + 40, j * 40 : (j + 1) * 40],
                            lhsT=k_v[r0 : r0 + 40, c0 + j],
                            rhs=q_v[r0 : r0 + 40, c0 + j],
                            start=True,
                            stop=True,
                        )
                e_sb = ep.tile([128, 480], BF16, tag="e_sb", name="e_sb")
                nc.scalar.activation(
                    out=e_sb[0:104, : gn * 40], in_=s_ps[0:104, : gn * 40], func=EXP
                )
                return e_sb

            def av_stage(c0, gn, e_sb, alt=[0]):
                o_ps = ps_x.tile([128, 512], F32, tag="ps_x", name="o_ps")
                for j in range(gn):
                    for cc in range(2):
                        r0 = cc * 64
                        nc.tensor.matmul(
                            o_ps[r0 : r0 + 41, j * 40 : (j + 1) * 40],
                            lhsT=vv[r0 : r0 + 40, c0 + j],
                            rhs=e_sb[r0 : r0 + 40, j * 40 : (j + 1) * 40],
                            start=True,
                            stop=True,
                        )
                nc.vector.tensor_copy(
                    out=o_all[0:105, c0 * 40 : c0 * 40 + gn * 40],
                    in_=o_ps[0:105, : gn * 40],
                )

            pend = [None]
            chunks = []
            for c0, gn in groups:
                def ch(c0=c0, gn=gn):
                    e_sb = scores_stage(c0, gn)
                    if pend[0] is not None:
                        av_stage(*pend[0])
                    pend[0] = (c0, gn, e_sb)
                chunks.append(ch)
            chunks.append(lambda: av_stage(*pend[0]))
            return o_all, chunks

        def pbwp_emit(o_all, i):
            branch_sb = brp.tile([128, 2, WD], BF16, name="branch_sb")
            out_sb = outp.tile([128, 2, WD], BF16, name="out_sb")
            o_v = o_all.rearrange("p (c w) -> p w c", w=W)
            chunks = []
            for wb in range(5):
                def ch(wb=wb):
                    pb_full = ps_x.tile([128, 1024], BF16, tag="ps_x", name="pb")
                    pb = pb_full[:, 0:848]
                    for wl in range(8):
                        w = wb * 8 + wl
                        nc.tensor.transpose(
                            pb[:, wl * 106 : wl * 106 + 105],
                            o_v[0:105, w, :],
                            ident[0:105, 0:105],
                        )
                    pb_v = pb.rearrange("p (w q) -> p w q", q=106)
                    rec = recp.tile([128, 8, 2], F32, tag="rec", name="rec")
                    nc.vector.reciprocal(rec[:, :, 0], pb_v[:, :, 40])
                    nc.vector.reciprocal(rec[:, :, 1], pb_v[:, :, 104])
                    for cc in range(2):
                        nc.vector.tensor_tensor(
                            branch_sb[:, cc].rearrange("p (w d) -> p w d", d=40)[
                                :, wb * 8 : wb * 8 + 8
                            ],
                            pb_v[:, :, cc * 64 : cc * 64 + 40],
                            rec[:, :, cc : cc + 1].to_broadcast((128, 8, 40)),
                            MULT,
                        )
                chunks.append(ch)
            for m in range(2):
                for n2 in range(2):
                    def ch(m=m, n2=n2):
                        for g in range(2):
                            n = n2 * 2 + g
                            pt = ps_cs.tile(
                                [128, 512], F32, tag="ps_cs", name="wp_ps"
                            )
                            ps = pt[:, 0:400]
                            for k in range(2):
                                nc.tensor.matmul(
                                    ps[:],
                                    lhsT=wp_sb[:, k, m * 128 : (m + 1) * 128],
                                    rhs=branch_sb[:, k, n * 400 : (n + 1) * 400],
                                    start=(k == 0),
                                    stop=(k == 1),
                                )
                            nc.scalar.activation(
                                out=out_sb[:, m, n * 400 : (n + 1) * 400],
                                in_=ps[:],
                                func=IDENT,
                                bias=bp_sb[:, m],
                                scale=1.0,
                            )
                    chunks.append(ch)

            out_dv = out_d.ap()[:, i * WD : (i + 1) * WD].rearrange(
                "(ko ki) n -> ki ko n", ki=128
            )
            if i == SLAB - 1:
                # split the final store so the m=0 half overlaps m=1's evacs
                def dma_m(m):
                    nc.scalar.dma_start(out_dv[:, m : m + 1], out_sb[:, m : m + 1])

                chunks.insert(7, lambda: dma_m(0))
                chunks.append(lambda: dma_m(1))
            else:
                def dma_ch():
                    nc.scalar.dma_start(out_dv, out_sb[:])

                chunks.append(dma_ch)
            return chunks

        # ---- software-pipelined slice loop, conv two rows ahead:
        # iter i runs attn(i) | conv(i+2) | pbwp(i-1), with xbar(i+1)
        # issued at iter start (its qkv slab was filled during iter i-1,
        # so the transpose has a full row of slack before attn(i+1))
        conv_ch = conv_emit(x0, qkv_bufs[0])
        att_cur, piv0_ch = xbar_emit(qkv_bufs[0], 0)
        for ch in conv_ch:
            ch()
        for ch in _merge(piv0_ch, conv_emit(x1, qkv_bufs[1])):
            ch()
        x2 = load_x(2)
        _x_hold = [x2]
        pending_pbwp = []
        att_nxt = None
        for i in range(SLAB):
            if i + 1 < SLAB:
                att_nxt, piv_ch = xbar_emit(qkv_bufs[(i + 1) % 2], (i + 1) % 2)
            else:
                att_nxt, piv_ch = None, []
            if i + 2 < SLAB:
                x_nxt = _x_hold[0]
                if i + 3 < SLAB:
                    _x_hold[0] = load_x(i + 3)
                other = conv_emit(x_nxt, qkv_bufs[(i + 2) % 2])
            else:
                other = []
            o_all, attn_ch = attn_emit(att_cur)
            # pbwp early (inputs ready at row start), conv late; pivots of the
            # next row spread throughout
            rest = _merge(piv_ch, list(pending_pbwp) + list(other))
            skew = min(2, len(rest))
            for ch in rest[:skew]:
                ch()
            for ch in _merge(attn_ch, rest[skew:]):
                ch()
            pending_pbwp = pbwp_emit(o_all, i)
            att_cur = att_nxt
        for ch in pending_pbwp:
            ch()

    nc.compile()
    return nc


_NC_CACHE = None


def _get_nc():
    global _NC_CACHE
    if _NC_CACHE is None:
        _NC_CACHE = _build_nc()
    return _NC_CACHE


def make_in_maps(x, wq, bq, wk, bk, wv, bv, wp, bp):
    bf = ml_dtypes.bfloat16
    wqkv = np.concatenate(
        [wq.T * SCALE, wk.T, wv.T], axis=1
    ).astype(bf)  # [C, 3C], lhsT layout (c_in rows, c_out cols)
    bqkv = np.concatenate([bq * SCALE, bk, bv]).reshape(3 * C, 1).astype(np.float32)
    wp3 = (3.0 * wp).T.astype(bf)  # [C, C]
    bp_ = bp.reshape(C, 1).astype(np.float32)
    in_maps = []
    for core in range(N_CORES):
        b = core // 4
        r0 = (core % 4) * SLAB
        x_slab = np.ascontiguousarray(
            x[b, :, r0 : r0 + SLAB].reshape(C, NSLAB)
        ).astype(bf)
        in_maps.append(
            {"x": x_slab, "wqkv": wqkv, "bqkv": bqkv, "wp3": wp3, "bp": bp_}
        )
    return in_maps


def run_on_cores(in_maps, **kw):
    nc = _get_nc()
    return run_bass_kernel_spmd(nc, in_maps, core_ids=list(range(N_CORES)), **kw)


def kernel(x, wq, bq, wk, bk, wv, bv, wp, bp):
    x = np.asarray(x, dtype=np.float32)
    in_maps = make_in_maps(
        x,
        np.asarray(wq, np.float32),
        np.asarray(bq, np.float32),
        np.asarray(wk, np.float32),
        np.asarray(bk, np.float32),
        np.asarray(wv, np.float32),
        np.asarray(bv, np.float32),
        np.asarray(wp, np.float32),
        np.asarray(bp, np.float32),
    )
    res = run_on_cores(in_maps)
    out = np.empty((B, C, H, W, D), np.float32)
    for core in range(N_CORES):
        b = core // 4
        r0 = (core % 4) * SLAB
        out[b, :, r0 : r0 + SLAB] = (
            res.results[core]["out"].astype(np.float32).reshape(C, SLAB, W, D)
        )
    return out


# revision 8
# speedup vs baseline: 1.0080x; 1.0013x over previous
"""AxialAttention Trainium2 kernel (8 NeuronCores, SPMD) — v2.

Sharding: core = b*4 + q; each core: one batch element, 10-row H-slab, all 256
channels. out = 3 * branch (branches identical); factor 3 folded into wp,
attention scale folded into wq/bq.

v2 vs v1: the q/k/v pivot transposes (PE, 307k cyc) and their psum->SBUF
copies (Act/DVE) are replaced by XBAR DMA transposes (InstDmaTransposeAnt,
14ns/tile on the mostly-idle DMA engines). The conv evacuates q,k into
w-major 48-padded slabs and v into a d-major 48-padded slab (the xbar
semantics out[p,m,l] = in[l, m*48+p] make the pad columns land on junk
partitions 40-47/104-111). Attention reads the same [d|w' @ deck-band,
(w|d, c)] layout as v1. AV psum is bf16 (2x DVE evac); output store is bf16.
"""

import sys

sys.path.insert(0, "/opt/trn_rl_repo")

import numpy as np
import ml_dtypes
from contextlib import ExitStack

import concourse.bass as bass
import concourse.tile as tile
from concourse import bacc, mybir
from concourse.bass_utils import run_bass_kernel_spmd
from concourse.masks import make_identity

BF16 = mybir.dt.bfloat16
F32 = mybir.dt.float32

B, C, H, W, D = 2, 256, 40, 40, 40
HEADS = 8
HD = C // HEADS
SCALE = HD ** -0.5
N_CORES = 8
SLAB = H // 4          # 10 H-rows per core
WD = W * D             # 1600
NSLAB = SLAB * WD      # 16000
CHALF = 128
PADW = 48              # xbar pad: free = 40 * 48, junk -> partitions 40-47


def _merge(a, b):
    """Proportionally interleave two chunk lists, preserving each order."""
    out = []
    na, nb = len(a), len(b)
    ia = ib = 0
    while ia < na or ib < nb:
        if ib >= nb or (ia * (nb + 1) <= ib * (na + 1) and ia < na):
            out.append(a[ia])
            ia += 1
        else:
            out.append(b[ib])
            ib += 1
    return out


def _build_nc():
    nc = bacc.Bacc(
        "TRN2",
        target_bir_lowering=False,
        debug=False,
        num_devices=N_CORES,
    )
    x_d = nc.declare_dram_parameter("x", [C, NSLAB], BF16, isOutput=False)
    wqkv_d = nc.declare_dram_parameter("wqkv", [C, 3 * C], BF16, isOutput=False)
    bqkv_d = nc.declare_dram_parameter("bqkv", [3 * C, 1], F32, isOutput=False)
    wp_d = nc.declare_dram_parameter("wp3", [C, C], BF16, isOutput=False)
    bp_d = nc.declare_dram_parameter("bp", [C, 1], F32, isOutput=False)
    out_d = nc.declare_dram_parameter("out", [C, NSLAB], BF16, isOutput=True)

    IDENT = mybir.ActivationFunctionType.Identity
    EXP = mybir.ActivationFunctionType.Exp
    MULT = mybir.AluOpType.mult

    with ExitStack() as ctx:
        tc = ctx.enter_context(tile.TileContext(nc))
        const = ctx.enter_context(tc.tile_pool(name="const", bufs=1))
        xp = ctx.enter_context(tc.tile_pool(name="xp", bufs=2))
        attp = ctx.enter_context(tc.tile_pool(name="attp", bufs=2))
        ep = ctx.enter_context(tc.tile_pool(name="ep", bufs=3))
        oallp = ctx.enter_context(tc.tile_pool(name="oallp", bufs=2))
        brp = ctx.enter_context(tc.tile_pool(name="brp", bufs=2))
        recp = ctx.enter_context(tc.tile_pool(name="recp", bufs=4))
        outp = ctx.enter_context(tc.tile_pool(name="outp", bufs=2))
        # conv+wp 2-bank tiles; scores tiles; o_ps/pb share the last 2 banks
        ps_cs = ctx.enter_context(tc.tile_pool(name="ps_cs", bufs=2, space="PSUM"))
        ps_s = ctx.enter_context(tc.tile_pool(name="ps_s", bufs=2, space="PSUM"))
        ps_x = ctx.enter_context(tc.tile_pool(name="ps_x", bufs=2, space="PSUM"))
        ps_t = ctx.enter_context(tc.tile_pool(name="ps_t", bufs=2, space="PSUM"))

        ident = const.tile([128, 128], BF16)
        make_identity(nc, ident[:])

        def load_x(i):
            x_sb = xp.tile([128, 2, WD], BF16, name="x_sb")
            nc.sync.dma_start(
                x_sb[:],
                x_d.ap()[:, i * WD : (i + 1) * WD].rearrange(
                    "(ko ki) n -> ki ko n", ki=128
                ),
            )
            return x_sb

        # DMA order: wqkv-q (tiny) then x0 halves, so the first Ldweights
        # and first conv matmul start as early as possible
        wqkv_sb = const.tile([128, 2, 3 * C], BF16)
        nc.sync.dma_start(
            wqkv_sb[:, :, 0:256],
            wqkv_d.ap()[:, 0:256].rearrange("(ko ki) m -> ki ko m", ki=128),
        )
        x0 = xp.tile([128, 2, WD], BF16, name="x_sb")
        x0_src = x_d.ap()[:, 0:WD].rearrange("(ko ki) n -> ki ko n", ki=128)
        nc.sync.dma_start(x0[:, :, 0:800], x0_src[:, :, 0:800])
        bqkv_sb = const.tile([128, 6, 1], F32)
        nc.sync.dma_start(
            bqkv_sb[:], bqkv_d.ap().rearrange("(mo mi) one -> mi mo one", mi=128)
        )
        nc.sync.dma_start(x0[:, :, 800:WD], x0_src[:, :, 800:WD])
        x1 = load_x(1)
        nc.sync.dma_start(
            wqkv_sb[:, :, 256 : 3 * C],
            wqkv_d.ap()[:, 256 : 3 * C].rearrange(
                "(ko ki) m -> ki ko m", ki=128
            ),
        )
        wp_sb = const.tile([128, 2, C], BF16)
        nc.sync.dma_start(
            wp_sb[:], wp_d.ap().rearrange("(ko ki) m -> ki ko m", ki=128)
        )
        bp_sb = const.tile([128, 2, 1], F32)
        nc.sync.dma_start(
            bp_sb[:], bp_d.ap().rearrange("(mo mi) one -> mi mo one", mi=128)
        )

        # persistent double-buffered compact qkv slabs (w-major), flat with a
        # 1KB pad so 64-wide pivot reads (M=64 fills psum rows fully; the PE
        # cost is out-free-size, so the widening is free) stay in-bounds
        qkv_slab0 = const.tile([128, 6 * WD + 1024], BF16)
        qkv_slab1 = const.tile([128, 6 * WD + 1024], BF16)
        qkv_bufs = (qkv_slab0, qkv_slab1)
        nc.vector.memset(qkv_slab0[:, 6 * WD :], 0.0)
        nc.vector.memset(qkv_slab1[:, 6 * WD :], 0.0)


        # channel-pair groups (deck covers c_local and c_local+128)
        groups = []
        c0 = 0
        while c0 < CHALF:
            groups.append((c0, min(12, CHALF - c0)))
            c0 += 12

        def conv_emit(x_sb, qkv_sb):
            # qkv slab: [128, m(6), 40, 48] padded; q,k w-major (outer=w),
            # v d-major (outer=d). m: q0,q1,k0,k1,v0,v1 (out-channel decks).
            chunks = []
            for m in range(6):
                for n2 in range(2):
                    def ch(m=m, n2=n2):
                        for g in range(2):
                            n = n2 * 2 + g
                            pt = ps_cs.tile(
                                [128, 512], F32, tag="ps_cs", name="conv_ps"
                            )
                            ps = pt[:, 0:400]
                            for k in range(2):
                                nc.tensor.matmul(
                                    ps[:],
                                    lhsT=wqkv_sb[:, k, m * 128 : (m + 1) * 128],
                                    rhs=x_sb[:, k, n * 400 : (n + 1) * 400],
                                    start=(k == 0),
                                    stop=(k == 1),
                                )
                            dst = qkv_sb[
                                :, m * WD + n * 400 : m * WD + (n + 1) * 400
                            ]
                            if (m + n2) % 3 == 0:
                                nc.vector.tensor_scalar_add(
                                    dst, ps[:], bqkv_sb[:, m]
                                )
                            else:
                                nc.scalar.activation(
                                    out=dst,
                                    in_=ps[:],
                                    func=IDENT,
                                    bias=bqkv_sb[:, m],
                                    scale=1.0,
                                )
                    chunks.append(ch)
            return chunks

        def xbar_emit(qkv_sb, parity):
            # PE-transpose pivots: q/k/v [d|w' @ deck-band, (w|d, c)].
            # pst is bf16 psum, evacuated with DVE 2x or Act.
            q_att = attp.tile([128, W * CHALF], BF16, tag="q_att", name="q_att")
            k_att = attp.tile([128, W * CHALF], BF16, tag="k_att", name="k_att")
            v_att = attp.tile([128, 41 * CHALF], BF16, tag="v_att", name="v_att")
            v_view = qkv_sb[:, 0 : 6 * WD].rearrange(
                "p (m w d) -> p m d w", m=6, d=D
            )
            chunks = [
                lambda: nc.vector.memset(v_att[:, 40 * CHALF : 41 * CHALF], 1.0)
            ]
            for src, dst, eng in (
                (0, q_att, "v"),
                (2, k_att, "s"),
                (4, v_att, "v"),
            ):
                for wg in range(5):
                    def ch(wg=wg, src=src, dst=dst, eng=eng):
                        pst = ps_t.tile(
                            [128, 1024], BF16, tag="ps_t", name="pst"
                        )
                        for wl in range(8):
                            w = wg * 8 + wl
                            for cc in range(2):
                                r0 = cc * 64
                                if src == 4:
                                    in_ap = v_view[:, 4 + cc, w]
                                else:
                                    off = (src + cc) * WD + w * 40
                                    in_ap = qkv_sb[:, off : off + 40]
                                nc.tensor.transpose(
                                    pst[r0 : r0 + 40, wl * 128 : (wl + 1) * 128],
                                    in_ap,
                                    ident[:],
                                )
                        if eng == "s":
                            nc.scalar.copy(
                                dst[0:104, wg * 1024 : (wg + 1) * 1024],
                                pst[0:104, :],
                            )
                        else:
                            nc.vector.tensor_copy(
                                out=dst[0:104, wg * 1024 : (wg + 1) * 1024],
                                in_=pst[0:104, :],
                            )
                    chunks.append(ch)
            return (q_att, k_att, v_att), chunks

        def attn_emit(att):
            q_att, k_att, v_att = att
            k_v = k_att.rearrange("p (w c) -> p c w", c=CHALF)
            q_v = q_att.rearrange("p (w c) -> p c w", c=CHALF)
            vv = v_att.rearrange("p (d c) -> p c d", c=CHALF)
            o_all = oallp.tile([128, CHALF * W], BF16, name="o_all")

            def scores_stage(c0, gn):
                s_ps = ps_s.tile([128, 512], F32, tag="ps_s", name="s_ps")
                for j in range(gn):
                    for cc in range(2):
                        r0 = cc * 64
                        nc.tensor.matmul(
                            s_ps[r0 : r0 + 40, j * 40 : (j + 1) * 40],
                            lhsT=k_v[r0 : r0 + 40, c0 + j],
                            rhs=q_v[r0 : r0 + 40, c0 + j],
                            start=True,
                            stop=True,
                        )
                e_sb = ep.tile([128, 480], BF16, tag="e_sb", name="e_sb")
                nc.scalar.activation(
                    out=e_sb[0:104, : gn * 40], in_=s_ps[0:104, : gn * 40], func=EXP
                )
                return e_sb

            def av_stage(c0, gn, e_sb, alt=[0]):
                o_ps = ps_x.tile([128, 512], F32, tag="ps_x", name="o_ps")
                for j in range(gn):
                    for cc in range(2):
                        r0 = cc * 64
                        nc.tensor.matmul(
                            o_ps[r0 : r0 + 41, j * 40 : (j + 1) * 40],
                            lhsT=vv[r0 : r0 + 40, c0 + j],
                            rhs=e_sb[r0 : r0 + 40, j * 40 : (j + 1) * 40],
                            start=True,
                            stop=True,
                        )
                nc.vector.tensor_copy(
                    out=o_all[0:105, c0 * 40 : c0 * 40 + gn * 40],
                    in_=o_ps[0:105, : gn * 40],
                )

            pend = [None]
            chunks = []
            for c0, gn in groups:
                def ch(c0=c0, gn=gn):
                    e_sb = scores_stage(c0, gn)
                    if pend[0] is not None:
                        av_stage(*pend[0])
                    pend[0] = (c0, gn, e_sb)
                chunks.append(ch)
            chunks.append(lambda: av_stage(*pend[0]))
            return o_all, chunks

        def pbwp_emit(o_all, i):
            branch_sb = brp.tile([128, 2, WD], BF16, name="branch_sb")
            out_sb = outp.tile([128, 2, WD], BF16, name="out_sb")
            o_v = o_all.rearrange("p (c w) -> p w c", w=W)
            chunks = []
            for wb in range(5):
                def ch(wb=wb):
                    pb_full = ps_x.tile([128, 1024], BF16, tag="ps_x", name="pb")
                    pb = pb_full[:, 0:848]
                    for wl in range(8):
                        w = wb * 8 + wl
                        nc.tensor.transpose(
                            pb[:, wl * 106 : wl * 106 + 105],
                            o_v[0:105, w, :],
                            ident[0:105, 0:105],
                        )
                    pb_v = pb.rearrange("p (w q) -> p w q", q=106)
                    rec = recp.tile([128, 8, 2], F32, tag="rec", name="rec")
                    nc.vector.reciprocal(rec[:, :, 0], pb_v[:, :, 40])
                    nc.vector.reciprocal(rec[:, :, 1], pb_v[:, :, 104])
                    for cc in range(2):
                        nc.vector.tensor_tensor(
                            branch_sb[:, cc].rearrange("p (w d) -> p w d", d=40)[
                                :, wb * 8 : wb * 8 + 8
                            ],
                            pb_v[:, :, cc * 64 : cc * 64 + 40],
                            rec[:, :, cc : cc + 1].to_broadcast((128, 8, 40)),
                            MULT,
                        )
                chunks.append(ch)
            for m in range(2):
                for n2 in range(2):
                    def ch(m=m, n2=n2):
                        for g in range(2):
                            n = n2 * 2 + g
                            pt = ps_cs.tile(
                                [128, 512], F32, tag="ps_cs", name="wp_ps"
                            )
                            ps = pt[:, 0:400]
                            for k in range(2):
                                nc.tensor.matmul(
                                    ps[:],
                                    lhsT=wp_sb[:, k, m * 128 : (m + 1) * 128],
                                    rhs=branch_sb[:, k, n * 400 : (n + 1) * 400],
                                    start=(k == 0),
                                    stop=(k == 1),
                                )
                            nc.scalar.activation(
                                out=out_sb[:, m, n * 400 : (n + 1) * 400],
                                in_=ps[:],
                                func=IDENT,
                                bias=bp_sb[:, m],
                                scale=1.0,
                            )
                    chunks.append(ch)

            out_dv = out_d.ap()[:, i * WD : (i + 1) * WD].rearrange(
                "(ko ki) n -> ki ko n", ki=128
            )
            # interleave wp after its pb/TT deps: [pb0 pb1 pb2 wp00 wp10
            # pb3 pb4 wp01 wp11]; on the final row the store splits per deck
            o = [chunks[k] for k in (0, 1, 2, 5, 7, 3, 4, 6, 8)]
            if i == SLAB - 1:
                def dma_m(m):
                    nc.scalar.dma_start(out_dv[:, m : m + 1], out_sb[:, m : m + 1])

                o.insert(8, lambda: dma_m(0))
                o.append(lambda: dma_m(1))
            else:
                def dma_ch():
                    nc.scalar.dma_start(out_dv, out_sb[:])

                o.append(dma_ch)
            return o

        # ---- software-pipelined slice loop, conv two rows ahead:
        # iter i runs attn(i) | conv(i+2) | pbwp(i-1), with xbar(i+1)
        # issued at iter start (its qkv slab was filled during iter i-1,
        # so the transpose has a full row of slack before attn(i+1))
        conv_ch = conv_emit(x0, qkv_bufs[0])
        att_cur, piv0_ch = xbar_emit(qkv_bufs[0], 0)
        for ch in conv_ch:
            ch()
        for ch in _merge(piv0_ch, conv_emit(x1, qkv_bufs[1])):
            ch()
        x2 = load_x(2)
        _x_hold = [x2]
        pending_pbwp = []
        att_nxt = None
        for i in range(SLAB):
            if i + 1 < SLAB:
                att_nxt, piv_ch = xbar_emit(qkv_bufs[(i + 1) % 2], (i + 1) % 2)
            else:
                att_nxt, piv_ch = None, []
            if i + 2 < SLAB:
                x_nxt = _x_hold[0]
                if i + 3 < SLAB:
                    _x_hold[0] = load_x(i + 3)
                other = conv_emit(x_nxt, qkv_bufs[(i + 2) % 2])
            else:
                other = []
            o_all, attn_ch = attn_emit(att_cur)
            # pbwp early (inputs ready at row start), conv late; pivots of the
            # next row spread throughout
            rest = _merge(piv_ch, list(pending_pbwp) + list(other))
            skew = min(2, len(rest))
            for ch in rest[:skew]:
                ch()
            for ch in _merge(attn_ch, rest[skew:]):
                ch()
            pending_pbwp = pbwp_emit(o_all, i)
            att_cur = att_nxt
        for ch in pending_pbwp:
            ch()

    nc.compile()
    return nc


_NC_CACHE = None


def _get_nc():
    global _NC_CACHE
    if _NC_CACHE is None:
        _NC_CACHE = _build_nc()
    return _NC_CACHE


def make_in_maps(x, wq, bq, wk, bk, wv, bv, wp, bp):
    bf = ml_dtypes.bfloat16
    wqkv = np.concatenate(
        [wq.T * SCALE, wk.T, wv.T], axis=1
    ).astype(bf)  # [C, 3C], lhsT layout (c_in rows, c_out cols)
    bqkv = np.concatenate([bq * SCALE, bk, bv]).reshape(3 * C, 1).astype(np.float32)
    wp3 = (3.0 * wp).T.astype(bf)  # [C, C]
    bp_ = bp.reshape(C, 1).astype(np.float32)
    in_maps = []
    for core in range(N_CORES):
        b = core // 4
        r0 = (core % 4) * SLAB
        x_slab = np.ascontiguousarray(
            x[b, :, r0 : r0 + SLAB].reshape(C, NSLAB)
        ).astype(bf)
        in_maps.append(
            {"x": x_slab, "wqkv": wqkv, "bqkv": bqkv, "wp3": wp3, "bp": bp_}
        )
    return in_maps


def run_on_cores(in_maps, **kw):
    nc = _get_nc()
    return run_bass_kernel_spmd(nc, in_maps, core_ids=list(range(N_CORES)), **kw)


def kernel(x, wq, bq, wk, bk, wv, bv, wp, bp):
    x = np.asarray(x, dtype=np.float32)
    in_maps = make_in_maps(
        x,
        np.asarray(wq, np.float32),
        np.asarray(bq, np.float32),
        np.asarray(wk, np.float32),
        np.asarray(bk, np.float32),
        np.asarray(wv, np.float32),
        np.asarray(bv, np.float32),
        np.asarray(wp, np.float32),
        np.asarray(bp, np.float32),
    )
    res = run_on_cores(in_maps)
    out = np.empty((B, C, H, W, D), np.float32)
    for core in range(N_CORES):
        b = core // 4
        r0 = (core % 4) * SLAB
        out[b, :, r0 : r0 + SLAB] = (
            res.results[core]["out"].astype(np.float32).reshape(C, SLAB, W, D)
        )
    return out


# revision 9
# speedup vs baseline: 1.0118x; 1.0038x over previous
"""AxialAttention Trainium2 kernel (8 NeuronCores, SPMD) — v2.

Sharding: core = b*4 + q; each core: one batch element, 10-row H-slab, all 256
channels. out = 3 * branch (branches identical); factor 3 folded into wp,
attention scale folded into wq/bq.

v2 vs v1: the q/k/v pivot transposes (PE, 307k cyc) and their psum->SBUF
copies (Act/DVE) are replaced by XBAR DMA transposes (InstDmaTransposeAnt,
14ns/tile on the mostly-idle DMA engines). The conv evacuates q,k into
w-major 48-padded slabs and v into a d-major 48-padded slab (the xbar
semantics out[p,m,l] = in[l, m*48+p] make the pad columns land on junk
partitions 40-47/104-111). Attention reads the same [d|w' @ deck-band,
(w|d, c)] layout as v1. AV psum is bf16 (2x DVE evac); output store is bf16.
"""

import sys

sys.path.insert(0, "/opt/trn_rl_repo")

import numpy as np
import ml_dtypes
from contextlib import ExitStack

import concourse.bass as bass
import concourse.tile as tile
from concourse import bacc, mybir
from concourse.bass_utils import run_bass_kernel_spmd
from concourse.masks import make_identity

BF16 = mybir.dt.bfloat16
F32 = mybir.dt.float32

B, C, H, W, D = 2, 256, 40, 40, 40
HEADS = 8
HD = C // HEADS
SCALE = HD ** -0.5
N_CORES = 8
SLAB = H // 4          # 10 H-rows per core
WD = W * D             # 1600
NSLAB = SLAB * WD      # 16000
CHALF = 128
PADW = 48              # xbar pad: free = 40 * 48, junk -> partitions 40-47


def _merge(a, b):
    """Proportionally interleave two chunk lists, preserving each order."""
    out = []
    na, nb = len(a), len(b)
    ia = ib = 0
    while ia < na or ib < nb:
        if ib >= nb or (ia * (nb + 1) <= ib * (na + 1) and ia < na):
            out.append(a[ia])
            ia += 1
        else:
            out.append(b[ib])
            ib += 1
    return out


def _build_nc():
    nc = bacc.Bacc(
        "TRN2",
        target_bir_lowering=False,
        debug=False,
        num_devices=N_CORES,
    )
    x_d = nc.declare_dram_parameter("x", [C, NSLAB], BF16, isOutput=False)
    wqkv_d = nc.declare_dram_parameter("wqkv", [C, 3 * C], BF16, isOutput=False)
    bqkv_d = nc.declare_dram_parameter("bqkv", [3 * C, 1], F32, isOutput=False)
    wp_d = nc.declare_dram_parameter("wp3", [C, C], BF16, isOutput=False)
    bp_d = nc.declare_dram_parameter("bp", [C, 1], F32, isOutput=False)
    out_d = nc.declare_dram_parameter("out", [C, NSLAB], BF16, isOutput=True)

    IDENT = mybir.ActivationFunctionType.Identity
    EXP = mybir.ActivationFunctionType.Exp
    MULT = mybir.AluOpType.mult

    with ExitStack() as ctx:
        tc = ctx.enter_context(tile.TileContext(nc))
        const = ctx.enter_context(tc.tile_pool(name="const", bufs=1))
        xp = ctx.enter_context(tc.tile_pool(name="xp", bufs=2))
        attp = ctx.enter_context(tc.tile_pool(name="attp", bufs=2))
        ep = ctx.enter_context(tc.tile_pool(name="ep", bufs=3))
        oallp = ctx.enter_context(tc.tile_pool(name="oallp", bufs=2))
        brp = ctx.enter_context(tc.tile_pool(name="brp", bufs=2))
        recp = ctx.enter_context(tc.tile_pool(name="recp", bufs=4))
        outp = ctx.enter_context(tc.tile_pool(name="outp", bufs=2))
        # conv+wp 2-bank tiles; scores tiles; o_ps/pb share the last 2 banks
        ps_cs = ctx.enter_context(tc.tile_pool(name="ps_cs", bufs=2, space="PSUM"))
        ps_s = ctx.enter_context(tc.tile_pool(name="ps_s", bufs=2, space="PSUM"))
        ps_x = ctx.enter_context(tc.tile_pool(name="ps_x", bufs=2, space="PSUM"))
        ps_t = ctx.enter_context(tc.tile_pool(name="ps_t", bufs=2, space="PSUM"))

        ident = const.tile([128, 128], BF16)
        make_identity(nc, ident[:])

        def load_x(i):
            x_sb = xp.tile([128, 2, WD], BF16, name="x_sb")
            nc.sync.dma_start(
                x_sb[:],
                x_d.ap()[:, i * WD : (i + 1) * WD].rearrange(
                    "(ko ki) n -> ki ko n", ki=128
                ),
            )
            return x_sb

        # DMA order: wqkv-q (tiny) then x0 halves, so the first Ldweights
        # and first conv matmul start as early as possible
        wqkv_sb = const.tile([128, 2, 3 * C], BF16)
        nc.sync.dma_start(
            wqkv_sb[:, :, 0:256],
            wqkv_d.ap()[:, 0:256].rearrange("(ko ki) m -> ki ko m", ki=128),
        )
        x0 = xp.tile([128, 2, WD], BF16, name="x_sb")
        x0_src = x_d.ap()[:, 0:WD].rearrange("(ko ki) n -> ki ko n", ki=128)
        nc.sync.dma_start(x0[:, :, 0:800], x0_src[:, :, 0:800])
        bqkv_sb = const.tile([128, 6, 1], F32)
        nc.sync.dma_start(
            bqkv_sb[:], bqkv_d.ap().rearrange("(mo mi) one -> mi mo one", mi=128)
        )
        nc.sync.dma_start(x0[:, :, 800:WD], x0_src[:, :, 800:WD])
        x1 = load_x(1)
        nc.sync.dma_start(
            wqkv_sb[:, :, 256 : 3 * C],
            wqkv_d.ap()[:, 256 : 3 * C].rearrange(
                "(ko ki) m -> ki ko m", ki=128
            ),
        )
        wp_sb = const.tile([128, 2, C], BF16)
        nc.sync.dma_start(
            wp_sb[:], wp_d.ap().rearrange("(ko ki) m -> ki ko m", ki=128)
        )
        bp_sb = const.tile([128, 2, 1], F32)
        nc.sync.dma_start(
            bp_sb[:], bp_d.ap().rearrange("(mo mi) one -> mi mo one", mi=128)
        )

        # persistent double-buffered compact qkv slabs (w-major), flat with a
        # 1KB pad so 64-wide pivot reads (M=64 fills psum rows fully; the PE
        # cost is out-free-size, so the widening is free) stay in-bounds
        qkv_slab0 = const.tile([128, 6 * WD + 1024], BF16)
        qkv_slab1 = const.tile([128, 6 * WD + 1024], BF16)
        qkv_bufs = (qkv_slab0, qkv_slab1)
        nc.vector.memset(qkv_slab0[:, 6 * WD :], 0.0)
        nc.vector.memset(qkv_slab1[:, 6 * WD :], 0.0)


        # channel-pair groups (deck covers c_local and c_local+128)
        groups = []
        c0 = 0
        while c0 < CHALF:
            groups.append((c0, min(12, CHALF - c0)))
            c0 += 12

        def conv_emit(x_sb, qkv_sb):
            # qkv slab: [128, m(6), 40, 48] padded; q,k w-major (outer=w),
            # v d-major (outer=d). m: q0,q1,k0,k1,v0,v1 (out-channel decks).
            chunks = []
            for m in range(6):
                for n2 in range(2):
                    def ch(m=m, n2=n2):
                        for g in range(2):
                            n = n2 * 2 + g
                            pt = ps_cs.tile(
                                [128, 512], F32, tag="ps_cs", name="conv_ps"
                            )
                            ps = pt[:, 0:400]
                            for k in range(2):
                                nc.tensor.matmul(
                                    ps[:],
                                    lhsT=wqkv_sb[:, k, m * 128 : (m + 1) * 128],
                                    rhs=x_sb[:, k, n * 400 : (n + 1) * 400],
                                    start=(k == 0),
                                    stop=(k == 1),
                                )
                            dst = qkv_sb[
                                :, m * WD + n * 400 : m * WD + (n + 1) * 400
                            ]
                            if (m + n2) % 3 == 0:
                                nc.vector.tensor_scalar_add(
                                    dst, ps[:], bqkv_sb[:, m]
                                )
                            else:
                                nc.scalar.activation(
                                    out=dst,
                                    in_=ps[:],
                                    func=IDENT,
                                    bias=bqkv_sb[:, m],
                                    scale=1.0,
                                )
                    chunks.append(ch)
            return chunks

        def xbar_emit(qkv_sb, parity):
            # PE-transpose pivots: q/k/v [d|w' @ deck-band, (w|d, c)].
            # pst is bf16 psum, evacuated with DVE 2x or Act.
            q_att = attp.tile([128, W * CHALF], BF16, tag="q_att", name="q_att")
            k_att = attp.tile([128, W * CHALF], BF16, tag="k_att", name="k_att")
            v_att = attp.tile([128, 41 * CHALF], BF16, tag="v_att", name="v_att")
            v_view = qkv_sb[:, 0 : 6 * WD].rearrange(
                "p (m w d) -> p m d w", m=6, d=D
            )
            chunks = [
                lambda: nc.vector.memset(v_att[:, 40 * CHALF : 41 * CHALF], 1.0)
            ]
            for src, dst, eng in (
                (0, q_att, "v"),
                (2, k_att, "s"),
                (4, v_att, "v"),
            ):
                for wg in range(5):
                    def ch(wg=wg, src=src, dst=dst, eng=eng):
                        pst = ps_t.tile(
                            [128, 1024], BF16, tag="ps_t", name="pst"
                        )
                        for wl in range(8):
                            w = wg * 8 + wl
                            for cc in range(2):
                                r0 = cc * 64
                                if src == 4:
                                    in_ap = v_view[:, 4 + cc, w]
                                else:
                                    off = (src + cc) * WD + w * 40
                                    in_ap = qkv_sb[:, off : off + 40]
                                nc.tensor.transpose(
                                    pst[r0 : r0 + 40, wl * 128 : (wl + 1) * 128],
                                    in_ap,
                                    ident[:],
                                )
                        if eng == "s":
                            nc.scalar.copy(
                                dst[0:104, wg * 1024 : (wg + 1) * 1024],
                                pst[0:104, :],
                            )
                        else:
                            nc.vector.tensor_copy(
                                out=dst[0:104, wg * 1024 : (wg + 1) * 1024],
                                in_=pst[0:104, :],
                            )
                    chunks.append(ch)
            return (q_att, k_att, v_att), chunks

        def attn_emit(att):
            q_att, k_att, v_att = att
            k_v = k_att.rearrange("p (w c) -> p c w", c=CHALF)
            q_v = q_att.rearrange("p (w c) -> p c w", c=CHALF)
            vv = v_att.rearrange("p (d c) -> p c d", c=CHALF)
            o_all = oallp.tile([128, CHALF * W], BF16, name="o_all")

            def scores_stage(c0, gn):
                s_ps = ps_s.tile([128, 512], F32, tag="ps_s", name="s_ps")
                for j in range(gn):
                    for cc in range(2):
                        r0 = cc * 64
                        nc.tensor.matmul(
                            s_ps[r0 : r0 + 40, j * 40 : (j + 1) * 40],
                            lhsT=k_v[r0 : r0 + 40, c0 + j],
                            rhs=q_v[r0 : r0 + 40, c0 + j],
                            start=True,
                            stop=True,
                        )
                e_sb = ep.tile([128, 480], BF16, tag="e_sb", name="e_sb")
                nc.scalar.activation(
                    out=e_sb[0:104, : gn * 40], in_=s_ps[0:104, : gn * 40], func=EXP
                )
                return e_sb

            def av_stage(c0, gn, e_sb, alt=[0]):
                o_ps = ps_x.tile([128, 512], F32, tag="ps_x", name="o_ps")
                for j in range(gn):
                    for cc in range(2):
                        r0 = cc * 64
                        nc.tensor.matmul(
                            o_ps[r0 : r0 + 41, j * 40 : (j + 1) * 40],
                            lhsT=vv[r0 : r0 + 40, c0 + j],
                            rhs=e_sb[r0 : r0 + 40, j * 40 : (j + 1) * 40],
                            start=True,
                            stop=True,
                        )
                nc.vector.tensor_copy(
                    out=o_all[0:105, c0 * 40 : c0 * 40 + gn * 40],
                    in_=o_ps[0:105, : gn * 40],
                )

            pend = [None]
            chunks = []
            for c0, gn in groups:
                def ch(c0=c0, gn=gn):
                    e_sb = scores_stage(c0, gn)
                    if pend[0] is not None:
                        av_stage(*pend[0])
                    pend[0] = (c0, gn, e_sb)
                chunks.append(ch)
            chunks.append(lambda: av_stage(*pend[0]))
            return o_all, chunks

        def pbwp_emit(o_all, i):
            branch_sb = brp.tile([128, 2, WD], BF16, name="branch_sb")
            out_sb = outp.tile([128, 2, WD], BF16, name="out_sb")
            o_v = o_all.rearrange("p (c w) -> p w c", w=W)
            chunks = []
            for wb in range(5):
                def ch(wb=wb):
                    pb_full = ps_x.tile([128, 1024], BF16, tag="ps_x", name="pb")
                    pb = pb_full[:, 0:848]
                    for wl in range(8):
                        w = wb * 8 + wl
                        nc.tensor.transpose(
                            pb[:, wl * 106 : wl * 106 + 105],
                            o_v[0:105, w, :],
                            ident[0:105, 0:105],
                        )
                    pb_v = pb.rearrange("p (w q) -> p w q", q=106)
                    rec = recp.tile([128, 8, 2], F32, tag="rec", name="rec")
                    nc.vector.reciprocal(rec[:, :, 0], pb_v[:, :, 40])
                    nc.vector.reciprocal(rec[:, :, 1], pb_v[:, :, 104])
                    for cc in range(2):
                        nc.vector.tensor_tensor(
                            branch_sb[:, cc].rearrange("p (w d) -> p w d", d=40)[
                                :, wb * 8 : wb * 8 + 8
                            ],
                            pb_v[:, :, cc * 64 : cc * 64 + 40],
                            rec[:, :, cc : cc + 1].to_broadcast((128, 8, 40)),
                            MULT,
                        )
                chunks.append(ch)
            for m in range(2):
                for n2 in range(2):
                    def ch(m=m, n2=n2):
                        for g in range(2):
                            n = n2 * 2 + g
                            pt = ps_cs.tile(
                                [128, 512], F32, tag="ps_cs", name="wp_ps"
                            )
                            ps = pt[:, 0:400]
                            for k in range(2):
                                nc.tensor.matmul(
                                    ps[:],
                                    lhsT=wp_sb[:, k, m * 128 : (m + 1) * 128],
                                    rhs=branch_sb[:, k, n * 400 : (n + 1) * 400],
                                    start=(k == 0),
                                    stop=(k == 1),
                                )
                            nc.scalar.activation(
                                out=out_sb[:, m, n * 400 : (n + 1) * 400],
                                in_=ps[:],
                                func=IDENT,
                                bias=bp_sb[:, m],
                                scale=1.0,
                            )
                    chunks.append(ch)

            out_dv = out_d.ap()[:, i * WD : (i + 1) * WD].rearrange(
                "(ko ki) n -> ki ko n", ki=128
            )
            # interleave wp after its pb/TT deps: [pb0 pb1 pb2 wp00 wp10
            # pb3 pb4 wp01 wp11]; on the final row the store splits per deck
            o = [chunks[k] for k in (0, 1, 2, 5, 7, 3, 4, 6, 8)]
            if i == SLAB - 1:
                def dma_m(m):
                    nc.scalar.dma_start(out_dv[:, m : m + 1], out_sb[:, m : m + 1])

                o.insert(8, lambda: dma_m(0))
                o.append(lambda: dma_m(1))
            else:
                def dma_ch():
                    nc.scalar.dma_start(out_dv, out_sb[:])

                o.append(dma_ch)
            return o

        # ---- software-pipelined slice loop, conv two rows ahead:
        # iter i runs attn(i) | conv(i+2) | pbwp(i-1), with xbar(i+1)
        # issued at iter start (its qkv slab was filled during iter i-1,
        # so the transpose has a full row of slack before attn(i+1))
        conv_ch = conv_emit(x0, qkv_bufs[0])
        att_cur, piv0_ch = xbar_emit(qkv_bufs[0], 0)
        for ch in conv_ch:
            ch()
        for ch in _merge(piv0_ch, conv_emit(x1, qkv_bufs[1])):
            ch()
        x2 = load_x(2)
        _x_hold = [x2]
        pending_pbwp = []
        att_nxt = None
        for i in range(SLAB):
            if i + 1 < SLAB:
                att_nxt, piv_ch = xbar_emit(qkv_bufs[(i + 1) % 2], (i + 1) % 2)
            else:
                att_nxt, piv_ch = None, []
            if i + 2 < SLAB:
                x_nxt = _x_hold[0]
                if i + 3 < SLAB:
                    _x_hold[0] = load_x(i + 3)
                other = conv_emit(x_nxt, qkv_bufs[(i + 2) % 2])
            else:
                other = []
            o_all, attn_ch = attn_emit(att_cur)
            # pbwp early (inputs ready at row start), conv late; pivots of the
            # next row spread throughout
            rest = _merge(piv_ch, list(pending_pbwp) + list(other))
            skew = min(8, len(rest))
            for ch in rest[:skew]:
                ch()
            for ch in _merge(attn_ch, rest[skew:]):
                ch()
            pending_pbwp = pbwp_emit(o_all, i)
            att_cur = att_nxt
        for ch in pending_pbwp:
            ch()

    nc.compile()
    return nc


_NC_CACHE = None


def _get_nc():
    global _NC_CACHE
    if _NC_CACHE is None:
        _NC_CACHE = _build_nc()
    return _NC_CACHE


def make_in_maps(x, wq, bq, wk, bk, wv, bv, wp, bp):
    bf = ml_dtypes.bfloat16
    wqkv = np.concatenate(
        [wq.T * SCALE, wk.T, wv.T], axis=1
    ).astype(bf)  # [C, 3C], lhsT layout (c_in rows, c_out cols)
    bqkv = np.concatenate([bq * SCALE, bk, bv]).reshape(3 * C, 1).astype(np.float32)
    wp3 = (3.0 * wp).T.astype(bf)  # [C, C]
    bp_ = bp.reshape(C, 1).astype(np.float32)
    in_maps = []
    for core in range(N_CORES):
        b = core // 4
        r0 = (core % 4) * SLAB
        x_slab = np.ascontiguousarray(
            x[b, :, r0 : r0 + SLAB].reshape(C, NSLAB)
        ).astype(bf)
        in_maps.append(
            {"x": x_slab, "wqkv": wqkv, "bqkv": bqkv, "wp3": wp3, "bp": bp_}
        )
    return in_maps


def run_on_cores(in_maps, **kw):
    nc = _get_nc()
    return run_bass_kernel_spmd(nc, in_maps, core_ids=list(range(N_CORES)), **kw)


def kernel(x, wq, bq, wk, bk, wv, bv, wp, bp):
    x = np.asarray(x, dtype=np.float32)
    in_maps = make_in_maps(
        x,
        np.asarray(wq, np.float32),
        np.asarray(bq, np.float32),
        np.asarray(wk, np.float32),
        np.asarray(bk, np.float32),
        np.asarray(wv, np.float32),
        np.asarray(bv, np.float32),
        np.asarray(wp, np.float32),
        np.asarray(bp, np.float32),
    )
    res = run_on_cores(in_maps)
    out = np.empty((B, C, H, W, D), np.float32)
    for core in range(N_CORES):
        b = core // 4
        r0 = (core % 4) * SLAB
        out[b, :, r0 : r0 + SLAB] = (
            res.results[core]["out"].astype(np.float32).reshape(C, SLAB, W, D)
        )
    return out
